# revision 8
# baseline (speedup 1.0000x reference)
"""GQA causal attention with RoPE, tensor-parallel over heads on 8 TRN2 NeuronCores.

Reference computation (per problem spec, all f32):
  q = rope(x @ Wq), k = rope(x @ Wk), v = x @ Wv    (GQA: 32 q heads, 8 kv heads, hd=64)
  out = softmax(causal(q k^T / 8)) v @ Wo

Sharding: core c owns q-heads 4c..4c+3 and kv-head c (column shards of
Wq/Wk/Wv).  Attention outputs (kept transposed, feature-major) are
AllGathered; the Wo projection is column-split: core c computes
out[:, 256c:256(c+1)] with the full gathered activations, so the final
output assembles by concatenation with no AllReduce.

Compute dtype on the TensorEngine is bf16 (f32 accumulation in PSUM);
softmax runs in f32 on the scalar/vector engines.
"""

import os
import sys

import numpy as np

for _p in ("/opt/trn_rl_repo",):
    if os.path.isdir(_p) and _p not in sys.path:
        sys.path.insert(0, _p)

from contextlib import ExitStack

import concourse.bass as bass
import concourse.tile as tile
from concourse import bacc, mybir
from concourse.bass_utils import run_bass_kernel_spmd

B, S, HID = 2, 2048, 2048
NH, NKV, HD = 32, 8, 64
TP = 8
QH = NH // TP          # 4 q heads per core
T = B * S              # 4096 tokens
QF = QH * HD           # 256 q features per core
OC = HID // TP         # 256 out cols per core
TOKC = 512             # token chunk for projection / q-chunk for attention
NHB = HID // 128       # 16 hid blocks

F32 = mybir.dt.float32
BF = mybir.dt.bfloat16

LAST_RESULTS = None
_NC_CACHE = None


def build_nc():
    nc = bacc.Bacc(None, target_bir_lowering=False)

    x = nc.declare_dram_parameter("x", [T, HID], F32, False)
    cos = nc.declare_dram_parameter("cos", [S, HD], F32, False)
    sin = nc.declare_dram_parameter("sin", [S, HD], F32, False)
    wq = nc.declare_dram_parameter("Wq", [HID, QF], F32, False)
    wk = nc.declare_dram_parameter("Wk", [HID, HD], F32, False)
    wv = nc.declare_dram_parameter("Wv", [HID, HD], F32, False)
    wo = nc.declare_dram_parameter("Wo", [HID, OC], F32, False)
    out = nc.declare_dram_parameter("out", [T, OC], F32, isOutput=True)

    with tile.TileContext(nc) as tc, ExitStack() as ctx:
        const = ctx.enter_context(tc.tile_pool(name="const", bufs=1))
        dram = ctx.enter_context(tc.tile_pool(name="dram", bufs=1, space="DRAM"))

        # ---- constants -------------------------------------------------
        ones128 = const.tile([128, 128], BF)
        nc.vector.memset(ones128[:], 1.0)
        ident = const.tile([128, 128], BF)
        # identity: keep ones where (p - f) == 0
        nc.gpsimd.affine_select(
            ident[:], ones128[:], pattern=[[-1, 128]], base=0,
            channel_multiplier=1, compare_op=mybir.AluOpType.is_equal, fill=0.0,
        )
        ones_col = const.tile([1, 64], BF)
        nc.vector.memset(ones_col[:], 1.0)
        id64hi = const.tile([128, 64], BF)
        nc.gpsimd.affine_select(
            id64hi[64:128, :], ones128[64:128, 0:64], pattern=[[-1, 64]], base=0,
            channel_multiplier=1, compare_op=mybir.AluOpType.is_equal, fill=0.0,
        )

        # ---- weights (bf16 casts) -------------------------------------
        wq_sb = []
        wkv_sb = []
        wo_sb = []
        for hb in range(NHB):
            t = const.tile([128, QF], BF, name=f"wq{hb}")
            nc.gpsimd.dma_start(t[:], wq[hb * 128:(hb + 1) * 128, :])
            wq_sb.append(t)
            t = const.tile([128, 128], BF, name=f"wkv{hb}")
            nc.gpsimd.dma_start(t[:, 0:HD], wk[hb * 128:(hb + 1) * 128, :])
            nc.gpsimd.dma_start(t[:, HD:128], wv[hb * 128:(hb + 1) * 128, :])
            wkv_sb.append(t)
            t = const.tile([128, OC], BF, name=f"wo{hb}")
            nc.gpsimd.dma_start(t[:], wo[hb * 128:(hb + 1) * 128, :])
            wo_sb.append(t)

        # ---- RoPE tables: cosT/sinTs [128, S] bf16 --------------------
        # rows 0..63 = cos^T (d-major); rows 64..127 duplicate (2 heads per tile)
        # sinTs rows 0..31 = -sin^T[0:32], rows 32..63 = +sin^T[32:64]
        cosT = const.tile([128, S], BF)
        sinTs = const.tile([128, S], BF)
        with tc.tile_pool(name="ropebld", bufs=4) as rb, \
             tc.tile_pool(name="ropeps", bufs=4, space="PSUM") as rp:
            for i in range(S // 128):
                cn = rb.tile([128, HD], BF, tag="cn")
                nc.gpsimd.dma_start(cn[:], cos[i * 128:(i + 1) * 128, :])
                ps = rp.tile([HD, 128], BF, tag="ps")
                nc.tensor.transpose(ps[:], cn[:], ident[:])
                nc.scalar.copy(cosT[0:HD, i * 128:(i + 1) * 128], ps[:])
                sn = rb.tile([128, HD], BF, tag="sn")
                nc.gpsimd.dma_start(sn[:], sin[i * 128:(i + 1) * 128, :])
                ps2 = rp.tile([HD, 128], BF, tag="ps2")
                nc.tensor.transpose(ps2[:], sn[:], ident[:])
                nc.scalar.mul(sinTs[0:32, i * 128:(i + 1) * 128], ps2[0:32, :], -1.0)
                nc.scalar.copy(sinTs[32:HD, i * 128:(i + 1) * 128], ps2[32:HD, :])
        nc.sync.dma_start(cosT[HD:128, :], cosT[0:HD, :])
        nc.sync.dma_start(sinTs[HD:128, :], sinTs[0:HD, :])

        # ---- collective buffers ---------------------------------------
        ag_in = dram.tile([QF, T], BF)
        ag_out = dram.tile([TP * QF, T], BF, addr_space="Shared")

        # ---- per-batch: projections + rope + attention ----------------
        NTC = S // TOKC  # 4 chunks per batch
        xa_pool = ctx.enter_context(tc.tile_pool(name="xa", bufs=4))
        xt_pool = ctx.enter_context(tc.tile_pool(name="xt", bufs=16))
        psum = ctx.enter_context(tc.tile_pool(name="psum", bufs=8, space="PSUM"))
        tp_ps = pj_ps = att_ps = pv_ps = bc_ps = psum
        qkv_pool = ctx.enter_context(tc.tile_pool(name="qkv", bufs=2))
        rope_pool = ctx.enter_context(tc.tile_pool(name="rope", bufs=1))
        v_pool = ctx.enter_context(tc.tile_pool(name="vtile", bufs=2 * (S // 128)))
        e_pool = ctx.enter_context(tc.tile_pool(name="epool", bufs=16))
        o_pool = ctx.enter_context(tc.tile_pool(name="opool", bufs=4))
        r_pool = ctx.enter_context(tc.tile_pool(name="rpool", bufs=4))

        for b in range(B):
            # -- QKV^T projection, token chunks of 512 --
            qt = [qkv_pool.tile([128, S], BF, tag=f"qt{i}", name=f"qt{i}") for i in range(2)]
            kvT = qkv_pool.tile([128, S], BF, tag="kvT")
            kdup = qkv_pool.tile([128, S], BF, tag="kdup")
            for tcn in range(NTC):
                xa = []
                for tt in range(4):
                    t = xa_pool.tile([128, HID], BF, tag="xa")
                    nc.gpsimd.dma_start(
                        t[:], x[b * S + tcn * TOKC + tt * 128:
                                b * S + tcn * TOKC + (tt + 1) * 128, :])
                    xa.append(t)
                xts = []
                for hb in range(NHB):
                    xt = xt_pool.tile([128, TOKC], BF, tag="xt")
                    for tt in range(4):
                        ps = tp_ps.tile([128, 128], BF, tag="ps")
                        nc.tensor.transpose(
                            ps[:], xa[tt][:, hb * 128:(hb + 1) * 128], ident[:])
                        nc.vector.tensor_copy(
                            xt[:, tt * 128:(tt + 1) * 128], ps[:])
                    xts.append(xt)
                psq0 = pj_ps.tile([128, TOKC], F32, tag="ps")
                psq1 = pj_ps.tile([128, TOKC], F32, tag="ps")
                pskv = pj_ps.tile([128, TOKC], F32, tag="ps")
                for hb in range(NHB):
                    st, sp = hb == 0, hb == NHB - 1
                    nc.tensor.matmul(psq0[:], wq_sb[hb][:, 0:128], xts[hb][:],
                                     start=st, stop=sp)
                    nc.tensor.matmul(psq1[:], wq_sb[hb][:, 128:256], xts[hb][:],
                                     start=st, stop=sp)
                    nc.tensor.matmul(pskv[:], wkv_sb[hb][:], xts[hb][:],
                                     start=st, stop=sp)
                cs = slice(tcn * TOKC, (tcn + 1) * TOKC)
                nc.vector.tensor_copy(qt[0][:, cs], psq0[:])
                nc.vector.tensor_copy(qt[1][:, cs], psq1[:])
                nc.vector.tensor_copy(kvT[:, cs], pskv[:])

            # -- RoPE on q (2 tiles, 2 heads each) and k --
            for qi in range(2):
                rot = rope_pool.tile([128, S], BF, tag="rot")
                for h2 in range(2):
                    o = h2 * 64
                    nc.sync.dma_start(rot[o:o + 32, :], qt[qi][o + 32:o + 64, :])
                    nc.sync.dma_start(rot[o + 32:o + 64, :], qt[qi][o:o + 32, :])
                tmp = rope_pool.tile([128, S], BF, tag="tmp")
                nc.vector.tensor_mul(tmp[:], qt[qi][:], cosT[:])
                nc.vector.tensor_mul(rot[:], rot[:], sinTs[:])
                nc.vector.tensor_add(qt[qi][:], tmp[:], rot[:])
            rotk = rope_pool.tile([HD, S], BF, tag="rotk")
            nc.sync.dma_start(rotk[0:32, :], kvT[32:HD, :])
            nc.sync.dma_start(rotk[32:HD, :], kvT[0:32, :])
            tmpk = rope_pool.tile([HD, S], BF, tag="tmpk")
            nc.vector.tensor_mul(tmpk[:], kvT[0:HD, :], cosT[0:HD, :])
            nc.vector.tensor_mul(rotk[:], rotk[:], sinTs[0:HD, :])
            nc.vector.tensor_add(kvT[0:HD, :], tmpk[:], rotk[:])
            nc.sync.dma_start(kdup[HD:128, :], kvT[0:HD, :])

            # -- V: transpose to token-major tiles [128, 65] (ones col) --
            vts = []
            for vb in range(S // 128):
                psv = tp_ps.tile([128, HD], BF, tag="ps")
                nc.tensor.transpose(
                    psv[:], kvT[HD:128, vb * 128:(vb + 1) * 128], id64hi[HD:128, :])
                vt_ = v_pool.tile([128, HD + 1], BF, tag="vt")
                nc.vector.tensor_copy(vt_[:, 0:HD], psv[:])
                nc.vector.memset(vt_[:, HD:HD + 1], 1.0)
                vts.append(vt_)

            # -- attention per head, q-chunks of 512 --
            for h in range(QH):
                r = h % 2
                qh_ap = qt[h // 2][r * 64:r * 64 + 64, :]
                k_src = kvT if r == 0 else kdup
                for qc in range(S // TOKC):
                    nkb = (qc + 1) * (TOKC // 128)
                    es = []
                    for kb in range(nkb):
                        psS = att_ps.tile([128, TOKC], F32, tag="ps")
                        nc.tensor.matmul(
                            psS[:], k_src[r * 64:r * 64 + 64, kb * 128:(kb + 1) * 128],
                            qh_ap[:, qc * TOKC:(qc + 1) * TOKC],
                            start=True, stop=True)
                        e = e_pool.tile([128, TOKC], BF, tag="e")
                        nc.scalar.activation(
                            e[:], psS[:], mybir.ActivationFunctionType.Exp,
                            scale=0.125)
                        if kb >= nkb - 4:
                            nc.gpsimd.affine_select(
                                e[:], e[:], pattern=[[1, TOKC]],
                                base=qc * TOKC - kb * 128,
                                channel_multiplier=-1,
                                compare_op=mybir.AluOpType.is_ge, fill=0.0)
                        es.append(e)
                    psO = pv_ps.tile([HD + 1, TOKC], F32, tag="ps")
                    for kb in range(nkb):
                        nc.tensor.matmul(psO[:], vts[kb][:], es[kb][:],
                                         start=(kb == 0), stop=(kb == nkb - 1))
                    recip = r_pool.tile([1, TOKC], F32, tag="recip")
                    nc.vector.reciprocal(recip[:], psO[HD:HD + 1, :])
                    recb = r_pool.tile([1, TOKC], BF, tag="recb")
                    nc.vector.tensor_copy(recb[:], recip[:])
                    psB = bc_ps.tile([HD, TOKC], F32, tag="ps")
                    nc.tensor.matmul(psB[:], ones_col[:], recb[:],
                                     start=True, stop=True)
                    bcs = o_pool.tile([HD, TOKC], BF, tag="bcs")
                    nc.vector.tensor_copy(bcs[:], psB[:])
                    ot = o_pool.tile([HD, TOKC], BF, tag="ot")
                    nc.vector.tensor_copy(ot[:], psO[0:HD, :])
                    at = o_pool.tile([HD, TOKC], BF, tag="at")
                    nc.vector.tensor_mul(at[:], ot[:], bcs[:])
                    nc.sync.dma_start(
                        ag_in[h * HD:(h + 1) * HD,
                              b * S + qc * TOKC:b * S + (qc + 1) * TOKC],
                        at[:])

        # ---- AllGather attention outputs ------------------------------
        nc.gpsimd.collective_compute(
            "AllGather", mybir.AluOpType.bypass,
            ins=[ag_in[:].opt()], outs=[ag_out[:].opt()],
            replica_groups=[list(range(TP))],
        )

        # ---- Wo projection (column slice), token quarters -------------
        with tc.tile_pool(name="ag_sb", bufs=32) as agp, \
             tc.tile_pool(name="wo_out", bufs=4) as woout:
            wops = psum
            TQ = 512
            for tq in range(T // TQ):
                agt = []
                for fb in range(NHB):
                    t = agp.tile([128, TQ], BF, tag="agt")
                    nc.sync.dma_start(
                        t[:], ag_out[fb * 128:(fb + 1) * 128,
                                     tq * TQ:(tq + 1) * TQ])
                    agt.append(t)
                for tb in range(TQ // 128):
                    psW = wops.tile([128, OC], F32, tag="ps")
                    for fb in range(NHB):
                        nc.tensor.matmul(
                            psW[:], agt[fb][:, tb * 128:(tb + 1) * 128],
                            wo_sb[fb][:], start=(fb == 0), stop=(fb == NHB - 1))
                    osb = woout.tile([128, OC], F32, tag="osb")
                    nc.vector.tensor_copy(osb[:], psW[:])
                    nc.sync.dma_start(
                        out[tq * TQ + tb * 128:tq * TQ + (tb + 1) * 128, :],
                        osb[:])

    nc.compile()
    return nc


def kernel(**inputs):
    global LAST_RESULTS, _NC_CACHE
    x = np.ascontiguousarray(inputs["x"].reshape(T, HID), dtype=np.float32)
    cos = np.ascontiguousarray(inputs["cos"], dtype=np.float32)
    sin = np.ascontiguousarray(inputs["sin"], dtype=np.float32)
    Wq = np.asarray(inputs["Wq"], dtype=np.float32)
    Wk = np.asarray(inputs["Wk"], dtype=np.float32)
    Wv = np.asarray(inputs["Wv"], dtype=np.float32)
    Wo = np.asarray(inputs["Wo"], dtype=np.float32)

    if _NC_CACHE is None:
        _NC_CACHE = build_nc()
    nc = _NC_CACHE

    in_maps = []
    for c in range(TP):
        in_maps.append({
            "x": x, "cos": cos, "sin": sin,
            "Wq": np.ascontiguousarray(Wq[:, c * QF:(c + 1) * QF]),
            "Wk": np.ascontiguousarray(Wk[:, c * HD:(c + 1) * HD]),
            "Wv": np.ascontiguousarray(Wv[:, c * HD:(c + 1) * HD]),
            "Wo": np.ascontiguousarray(Wo[:, c * OC:(c + 1) * OC]),
        })

    res = run_bass_kernel_spmd(nc, in_maps, core_ids=list(range(TP)))
    LAST_RESULTS = res
    full = np.concatenate([res.results[c]["out"] for c in range(TP)], axis=1)
    return np.ascontiguousarray(full.reshape(B, S, HID), dtype=np.float32)


if __name__ == "__main__":
    nc = build_nc()
    print("build OK, instructions:",
          sum(len(bb.instructions) for bb in nc.main_func.blocks))
